# revision 14
# baseline (speedup 1.0000x reference)
"""LoRA attention Bass kernel for 8x Trainium2 NeuronCores — v3.

Sharding (Megatron tensor-parallel over heads): each of the 8 cores owns
2 heads (128 projection columns); q/k/v column-sharded, out-proj
row-sharded, partials summed on host.  LoRA merged into base weights on
host.  All matmul operands bf16; PSUM accumulation f32.

v3 structure: phases are interleaved per (batch, chunk) so the ACT-bound
attention loop (exp) overlaps the PE-bound projection matmuls of the next
batch, keeping the tensor engine dense (HAM stays at full clock).

PSUM budget (8 banks):
  big (pv_a, pv_b, q, k, v rotate, 4 bufs) = 4
  s (2 bufs)                               = 2
  out-proj                                 = 1
  bc broadcast                             = 1
The shared "big" tag paces batch b+1's projections against batch b's
attention: each pair's v matmuls wait for a bank freed by the paired
chunk's normalize, so the projection matmuls spread across the whole
attention timeline as PE filler (keeps HAM at full clock).  The v
projection packs its 4 [128,128] column tiles into ONE psum bank as a
single accumulation group (start zeroes the whole bank region).
"""

import os
import numpy as np
import ml_dtypes

import concourse.bass as bass
import concourse.mybir as mybir
import concourse.tile as tile
from concourse import bacc
from concourse.bass_utils import run_bass_kernel_spmd

F32 = mybir.dt.float32
BF16 = mybir.dt.bfloat16
AF = mybir.ActivationFunctionType

N_CORES = 8

D_MODEL = 1024
N_HEADS = 16
D_K = 64
LORA_R = 8
SCALING = 2.0
B = 4
S = 2048


class Cfg:
    def __init__(self, b=B, s=S, d=D_MODEL, cpc=128, dk=D_K):
        self.b = b
        self.s = s
        self.d = d
        self.cpc = cpc
        self.dk = dk
        self.seq = b * s
        self.nkc = d // 128
        self.sc = 512
        self.nsc = self.seq // self.sc
        self.nt = s // 128             # t chunks per batch
        self.nsb = s // self.sc        # s chunks per batch


def _build_nc(cfg: Cfg):
    c = cfg
    nc = bacc.Bacc("TRN2", target_bir_lowering=False, debug=False,
                   num_devices=N_CORES)

    xT = nc.dram_tensor("xT", [c.d, c.seq], BF16, kind="ExternalInput").ap()
    wq = nc.dram_tensor("wq", [c.d, c.cpc], BF16, kind="ExternalInput").ap()
    wk = nc.dram_tensor("wk", [c.d, c.cpc], BF16, kind="ExternalInput").ap()
    wv = nc.dram_tensor("wv", [c.d, c.cpc], BF16, kind="ExternalInput").ap()
    wo = nc.dram_tensor("wo", [c.cpc, c.d], BF16, kind="ExternalInput").ap()
    bq = nc.dram_tensor("bq", [c.cpc, 1], F32, kind="ExternalInput").ap()
    bk = nc.dram_tensor("bk", [c.cpc, 1], F32, kind="ExternalInput").ap()
    out = nc.dram_tensor("out", [c.seq, c.d], F32, kind="ExternalOutput").ap()

    dk = c.dk
    nj = c.sc // 128

    with tile.TileContext(nc) as tc:
        with tc.tile_pool(name="persist", bufs=1) as persist, \
             tc.tile_pool(name="xin", bufs=10) as xpool, \
             tc.tile_pool(name="big", bufs=4, space="PSUM") as bigpool, \
             tc.tile_pool(name="sps", bufs=2, space="PSUM") as spool, \
             tc.tile_pool(name="ops", bufs=1, space="PSUM") as opool, \
             tc.tile_pool(name="bcps", bufs=1, space="PSUM") as bcpool, \
             tc.tile_pool(name="exp", bufs=6) as epool, \
             tc.tile_pool(name="norm", bufs=7) as npool, \
             tc.tile_pool(name="bcs", bufs=3) as bcspool, \
             tc.tile_pool(name="rec", bufs=3) as rpool, \
             tc.tile_pool(name="osb", bufs=4) as osbpool:

            # ---- persistent tensors ----
            # per-batch q/k transposed + v natural (separate tiles so the
            # scheduler can overlap batch b's attention with batch b+1's
            # projections)
            qT = [persist.tile([128, c.s], BF16, tag=f"qT{b}", name=f"qT{b}")
                  for b in range(c.b)]
            kT = [persist.tile([128, c.s], BF16, tag=f"kT{b}", name=f"kT{b}")
                  for b in range(c.b)]
            # v: [128, nt, 130]: 0:64 headA, 64 ones, 65:129 headB, 129 ones
            vN = [persist.tile([128, c.nt, 2 * dk + 2], BF16, tag=f"v{b}", name=f"v{b}")
                  for b in range(c.b)]
            wq_sb = persist.tile([128, c.nkc, c.cpc], BF16, tag="wq")
            wk_sb = persist.tile([128, c.nkc, c.cpc], BF16, tag="wk")
            wv_sb = persist.tile([128, c.nkc, c.cpc], BF16, tag="wv")
            wo_sb = persist.tile([c.cpc, c.d], BF16, tag="wo")
            bq_sb = persist.tile([c.cpc, 1], F32, tag="bq")
            bk_sb = persist.tile([c.cpc, 1], F32, tag="bk")

            nc.sync.dma_start(out=wq_sb[:], in_=wq.rearrange("(kc p) m -> p kc m", p=128))
            nc.sync.dma_start(out=wk_sb[:], in_=wk.rearrange("(kc p) m -> p kc m", p=128))
            nc.sync.dma_start(out=wv_sb[:], in_=wv.rearrange("(kc p) m -> p kc m", p=128))
            nc.sync.dma_start(out=wo_sb[:], in_=wo[:])
            nc.sync.dma_start(out=bq_sb[:], in_=bq[:])
            nc.sync.dma_start(out=bk_sb[:], in_=bk[:])

            ones_f32 = persist.tile([128, 1], F32, tag="ones_f32")
            nc.vector.memset(ones_f32[:], 1.0)
            for b in range(c.b):
                nc.vector.tensor_copy(
                    vN[b][:, :, dk:dk + 1],
                    ones_f32[:].unsqueeze(1).to_broadcast([128, c.nt, 1]))
                nc.vector.tensor_copy(
                    vN[b][:, :, 2 * dk + 1:2 * dk + 2],
                    ones_f32[:].unsqueeze(1).to_broadcast([128, c.nt, 1]))
            ones_sb = persist.tile([dk + 1, dk], BF16, tag="ones")
            nc.vector.tensor_copy(
                ones_sb[:], ones_f32[0:dk + 1, :].to_broadcast([dk + 1, dk]))

            def phase1_chunk(b, sb):
                """Projections for s-chunk sb of batch b."""
                s0 = sb * c.sc          # offset within batch
                g0 = b * c.s + s0       # global offset
                q_ps = bigpool.tile([128, c.sc], F32, tag="big",
                                    name=f"q_ps_{b}_{sb}")
                k_ps = bigpool.tile([128, c.sc], F32, tag="big",
                                    name=f"k_ps_{b}_{sb}")
                v_ps = bigpool.tile([128, c.sc], F32, tag="big",
                                    name=f"v_ps_{b}_{sb}")
                for kc in range(c.nkc):
                    x_t = xpool.tile([128, c.sc], BF16, tag="x")
                    nc.gpsimd.dma_start(
                        out=x_t[:],
                        in_=xT[kc * 128:(kc + 1) * 128, g0:g0 + c.sc])
                    st = (kc == 0)
                    sp = (kc == c.nkc - 1)
                    nc.tensor.matmul(q_ps[:], wq_sb[:, kc, :], x_t[:],
                                     start=st, stop=sp)
                    nc.tensor.matmul(k_ps[:], wk_sb[:, kc, :], x_t[:],
                                     start=st, stop=sp)
                    # 4 [128,128] regions of ONE bank, a single accumulation
                    # group: start=True eagerly zeroes the whole bank region,
                    # later matmuls accumulate (first write per region lands
                    # on the zeroed bank)
                    for j in range(nj):
                        nc.tensor.matmul(
                            v_ps[:, j * 128:(j + 1) * 128],
                            x_t[:, j * 128:(j + 1) * 128],
                            wv_sb[:, kc, :],
                            start=(st and j == 0),
                            stop=(sp and j == nj - 1))
                nc.scalar.activation(qT[b][:, s0:s0 + c.sc], q_ps[:],
                                     AF.Identity, bias=bq_sb[:])
                nc.scalar.activation(kT[b][:, s0:s0 + c.sc], k_ps[:],
                                     AF.Identity, bias=bk_sb[:])
                tc0 = sb * nj
                # two wide strided copies: [128, 4(j), 64] -> [128, 4(tc), 64]
                with nc.allow_low_precision(reason="v rounded to bf16"):
                    nc.vector.tensor_copy(
                        vN[b][:, tc0:tc0 + nj, 0:dk],
                        v_ps[:].rearrange("p (j m) -> p j m", j=nj)[:, :, 0:dk])
                    nc.vector.tensor_copy(
                        vN[b][:, tc0:tc0 + nj, dk + 1:2 * dk + 1],
                        v_ps[:].rearrange("p (j m) -> p j m", j=nj)[:, :, dk:2 * dk])

            def phase2_chunk(b, sb):
                """Attention + out-proj for s-chunk sb of batch b."""
                s0 = sb * c.sc
                g0 = b * c.s + s0
                pv_a = bigpool.tile([128, c.sc], F32, tag="big",
                                    name=f"pv_a_{b}_{sb}")
                pv_b = bigpool.tile([128, c.sc], F32, tag="big",
                                    name=f"pv_b_{b}_{sb}")
                for t in range(c.nt):
                    t0 = t * 128
                    # both heads' scores back-to-back: same 64x128 tiling
                    # mode (T0/T8 via base-partition auto-derive) so they
                    # run concurrently in the array; PV matmuls (128-mode)
                    # grouped after to halve tiling-mode switches
                    s_a = spool.tile([128, c.sc], F32, tag="s")
                    s_b = spool.tile([128, c.sc], F32, tag="s")
                    nc.tensor.matmul(
                        s_a[:], kT[b][0:dk, t0:t0 + 128],
                        qT[b][0:dk, s0:s0 + c.sc],
                        start=True, stop=True)
                    nc.tensor.matmul(
                        s_b[:], kT[b][dk:2 * dk, t0:t0 + 128],
                        qT[b][dk:2 * dk, s0:s0 + c.sc],
                        start=True, stop=True)
                    e_a = epool.tile([128, c.sc], BF16, tag="e")
                    nc.scalar.activation(e_a[:], s_a[:], AF.Exp,
                                         scale=1.0 / np.sqrt(dk))
                    e_b = epool.tile([128, c.sc], BF16, tag="e")
                    nc.scalar.activation(e_b[:], s_b[:], AF.Exp,
                                         scale=1.0 / np.sqrt(dk))
                    nc.tensor.matmul(
                        pv_a[0:dk + 1, :], vN[b][:, t, 0:dk + 1], e_a[:],
                        start=(t == 0), stop=(t == c.nt - 1))
                    nc.tensor.matmul(
                        pv_b[0:dk + 1, :], vN[b][:, t, dk + 1:2 * dk + 2], e_b[:],
                        start=(t == 0), stop=(t == c.nt - 1))

                # normalize via fast recip of denom row (row dk of pv)
                # denom row -> partition-0 SBUF, fast recip there (the
                # probe-validated pattern), cast to bf16 for the K=1
                # broadcast matmul
                den_a = rpool.tile([1, c.sc], F32, tag="da", name=f"den_a_{b}_{sb}")
                den_b = rpool.tile([1, c.sc], F32, tag="db", name=f"den_b_{b}_{sb}")
                nc.vector.tensor_copy(den_a[:], pv_a[dk:dk + 1, :])
                nc.vector.tensor_copy(den_b[:], pv_b[dk:dk + 1, :])
                rec_a = rpool.tile([1, c.sc], F32, tag="ra", name=f"rec_a_{b}_{sb}")
                rec_b = rpool.tile([1, c.sc], F32, tag="rb", name=f"rec_b_{b}_{sb}")
                nc.vector.reciprocal_approx_fast(rec_a[:], den_a[:])
                nc.vector.reciprocal_approx_fast(rec_b[:], den_b[:])
                recb_a = rpool.tile([1, c.sc], BF16, tag="rba", name=f"recb_a_{b}_{sb}")
                recb_b = rpool.tile([1, c.sc], BF16, tag="rbb", name=f"recb_b_{b}_{sb}")
                with nc.allow_low_precision(reason="recip to bf16 for matmul"):
                    nc.vector.tensor_copy(recb_a[:], rec_a[:])
                    nc.vector.tensor_copy(recb_b[:], rec_b[:])
                bc_a = bcpool.tile([dk, c.sc], F32, tag="bc")
                bc_b = bcpool.tile([dk, c.sc], F32, tag="bc")
                nc.tensor.matmul(bc_a[:], ones_sb[0:1, :],
                                 recb_a[:], start=True, stop=True)
                nc.tensor.matmul(bc_b[:], ones_sb[0:1, :],
                                 recb_b[:], start=True, stop=True)
                bcs_a = bcspool.tile([dk, c.sc], F32, tag="bcs")
                bcs_b = bcspool.tile([dk, c.sc], F32, tag="bcs")
                nc.vector.tensor_copy(bcs_a[:], bc_a[:])
                nc.vector.tensor_copy(bcs_b[:], bc_b[:])
                norm = npool.tile([2 * dk, c.sc], BF16, tag="norm")
                with nc.allow_low_precision(reason="attn out bf16 for out-proj"):
                    nc.vector.tensor_tensor(
                        norm[0:dk, :], pv_a[0:dk, :], bcs_a[:],
                        mybir.AluOpType.mult)
                    nc.vector.tensor_tensor(
                        norm[dk:2 * dk, :], pv_b[0:dk, :], bcs_b[:],
                        mybir.AluOpType.mult)

                return norm, g0

            def emit_outproj(norm, g0, b, sb, flush=False):
                """Out-projection for a (deferred) chunk: single K=128
                matmul per tile.  During flush the (now idle) s banks join
                the rotation so the final drain is not single-bank-bound."""
                ew = min(512, c.d)
                for j in range(c.sc // 128):
                    for e in range(c.d // ew):
                        o_ps = opool.tile([128, ew], F32, tag="o",
                                          name=f"o_ps_{b}_{sb}_{j}_{e}")
                        nc.tensor.matmul(
                            o_ps[:],
                            norm[:, j * 128:(j + 1) * 128],
                            wo_sb[:, e * ew:(e + 1) * ew],
                            start=True, stop=True)
                        o_t = osbpool.tile([128, ew], F32, tag="osb")
                        nc.vector.tensor_copy(o_t[:], o_ps[:])
                        nc.sync.dma_start(
                            out=out[g0 + j * 128:g0 + (j + 1) * 128,
                                    e * ew:(e + 1) * ew],
                            in_=o_t[:])

            # ---- emission: phase1(0), then round-robin so batch b's
            # attention interleaves with batch b+1's projections ----
            DEFER = 5
            pending = []
            for sb in range(c.nsb):
                phase1_chunk(0, sb)
            for b in range(c.b):
                for sb in range(c.nsb):
                    norm, g0 = phase2_chunk(b, sb)
                    pending.append((norm, g0, b, sb))
                    if b + 1 < c.b:
                        phase1_chunk(b + 1, sb)
                    if len(pending) > DEFER:
                        emit_outproj(*pending.pop(0))
            for item in pending:
                emit_outproj(*item, flush=True)

    nc.compile()
    return nc


_NC_CACHE = {}


def get_nc(cfg: Cfg | None = None):
    cfg = cfg or Cfg()
    key = (cfg.b, cfg.s, cfg.d, cfg.cpc, cfg.dk)
    if key not in _NC_CACHE:
        _NC_CACHE[key] = _build_nc(cfg)
    return _NC_CACHE[key]


def make_in_maps(inputs: dict, cfg: Cfg | None = None):
    """Merge LoRA into base weights, shard per-core, convert to bf16."""
    c = cfg or Cfg()
    bf16 = ml_dtypes.bfloat16

    def merge(w, a, u):
        return (np.asarray(w, np.float64)
                + (np.asarray(a, np.float64) @ np.asarray(u, np.float64))
                * SCALING)

    wq_eff = merge(inputs["w_q"], inputs["a_q"], inputs["u_q"]).astype(bf16)
    wk_eff = merge(inputs["w_k"], inputs["a_k"], inputs["u_k"]).astype(bf16)
    wv_eff = merge(inputs["w_v"], inputs["a_v"], inputs["u_v"]).astype(bf16)
    w_o = np.asarray(inputs["w_o"], np.float32).astype(bf16)
    b_q = np.asarray(inputs["b_q"], np.float32)
    b_k = np.asarray(inputs["b_k"], np.float32)
    x = np.asarray(inputs["x"], np.float32)
    xT = np.ascontiguousarray(x.reshape(c.seq, c.d).T.astype(bf16))

    in_maps = []
    for i in range(N_CORES):
        sl = slice(i * c.cpc, (i + 1) * c.cpc)
        in_maps.append({
            "xT": xT,
            "wq": np.ascontiguousarray(wq_eff[:, sl]),
            "wk": np.ascontiguousarray(wk_eff[:, sl]),
            "wv": np.ascontiguousarray(wv_eff[:, sl]),
            "wo": np.ascontiguousarray(w_o[sl, :]),
            "bq": np.ascontiguousarray(b_q[sl]).reshape(c.cpc, 1),
            "bk": np.ascontiguousarray(b_k[sl]).reshape(c.cpc, 1),
        })
    return in_maps


def kernel(x, w_q, b_q, w_k, b_k, w_v, b_v, w_o, b_o,
           a_q, u_q, a_k, u_k, a_v, u_v):
    cfg = Cfg()
    c = cfg
    inputs = dict(x=x, w_q=w_q, b_q=b_q, w_k=w_k, b_k=b_k, w_v=w_v, b_v=b_v,
                  w_o=w_o, b_o=b_o, a_q=a_q, u_q=u_q, a_k=a_k, u_k=u_k,
                  a_v=a_v, u_v=u_v)
    in_maps = make_in_maps(inputs, cfg)

    nc = get_nc(cfg)
    res = run_bass_kernel_spmd(nc, in_maps, list(range(N_CORES)))
    out = np.zeros((c.seq, c.d), np.float32)
    for i in range(N_CORES):
        out += res.results[i]["out"]
    b_v = np.asarray(b_v, np.float64)
    w_o_f = np.asarray(w_o, np.float64)
    b_o = np.asarray(b_o, np.float64)
    out += (b_v @ w_o_f + b_o).astype(np.float32)
    return out.reshape(B, S, D_MODEL).astype(np.float32)
